# revision 20
# baseline (speedup 1.0000x reference)
"""Trainium2 Bass kernel for a dense transformer block (B=8, N=1024, C=768, H=12).

Sharding: pure data-parallel over batch - core b computes batch element b.
No collectives. Host prepares per-core inputs (transposed k_conn, folded /
transposed weights in fp16) and reassembles the [8, 1024, 768] output.

v2 design vs the v1 baseline:
  - LN1 is folded into the qkv/v matmuls: raw x@W runs immediately (no wait
    on LN stats); the centering term is a rank-1 K=2 fixup matmul
    (-mu[n]*colsum_W[d] + std[n]*bias[d]) accumulated into the same PSUM,
    and the rstd[n] scale is fused into the PSUM evacuation (DVE multiply by
    a broadcast rstd row for feature-major qk; ACT copy with per-partition
    rstd scale for token-major v).
  - softmax denominators never touch DRAM: each head's PSUM [65,N] (64 out
    rows + ones-column sums row) is evacuated in ONE ACT copy; GpSimd
    broadcasts the sums row across partitions and divides in place. The
    ones-column is placed FIRST for odd heads so the GpSimd divide is
    partition-aligned (no cross-Q7-core shifts).
  - B-phase qk tiles are interleaved into phase D as PE filler so the PE
    stays busy (and in high p-state) while DVE (kc multiply) and ACT (exp)
    grind through the attention elementwise work.
  - LN2's center/scale runs on GpSimd; znT PSUM->SBUF copies on ACT;
    proj bias is folded into the host-side residual (x + proj_b).
"""

import os
import sys

import numpy as np

for _p in ("/opt/trn_rl_repo", "/root/.axon_site/_ro/trn_rl_repo"):
    if os.path.isdir(_p) and _p not in sys.path:
        sys.path.insert(0, _p)

import concourse.bass as bass
import concourse.bacc as bacc
import concourse.tile as tile
from concourse import mybir
from concourse.bass_utils import run_bass_kernel_spmd
from concourse.masks import make_identity

B, N, C, H = 8, 1024, 768, 12
HS = C // H                 # 64 head size
SCALE = HS ** -0.5
EPS = 1e-5
P = 128                     # partitions
NT = N // P                 # 8 token tiles
CC = C // P                 # 6 channel chunks
DT = (2 * C) // P           # 12 M-tiles covering q then k
VW = H * (HS + 1)           # 780: v columns with a ones-column per head
AF = mybir.ActivationFunctionType
OP = mybir.AluOpType
f32 = mybir.dt.float32
f16 = mybir.dt.float16


def build_kernel():
    nc = bacc.Bacc("TRN2", target_bir_lowering=False, debug=False,
                   enable_asserts=False)

    x_d = nc.declare_dram_parameter("x", [N, C], f16, isOutput=False)
    xT_d = nc.declare_dram_parameter("xT", [C, N], f16, isOutput=False)
    kcT_d = nc.declare_dram_parameter("kcT", [N, N], f16, isOutput=False)
    wqk_d = nc.declare_dram_parameter("wqkT", [C, 2 * C], f16, isOutput=False)
    rqk_d = nc.declare_dram_parameter("rows_qk", [2, 2 * C], f16, isOutput=False)
    wv_d = nc.declare_dram_parameter("wvT", [C, VW], f16, isOutput=False)
    rv_d = nc.declare_dram_parameter("rows_v", [2, VW], f16, isOutput=False)
    wp_d = nc.declare_dram_parameter("projT", [C, C], f16, isOutput=False)
    w1_d = nc.declare_dram_parameter("fc1T", [C, C], f16, isOutput=False)
    b1_d = nc.declare_dram_parameter("fc1_b", [C], f32, isOutput=False)
    w2_d = nc.declare_dram_parameter("fc2T", [C, C], f16, isOutput=False)
    b2_d = nc.declare_dram_parameter("fc2_b", [C], f16, isOutput=False)
    out_d = nc.declare_dram_parameter("out", [N, C], f32, isOutput=True)

    with tile.TileContext(nc) as tc:
        with (
            tc.tile_pool(name="consts", bufs=1) as consts,
            tc.tile_pool(name="acts", bufs=1) as acts,
            tc.tile_pool(name="tp", bufs=3) as tp,
            tc.tile_pool(name="ps", bufs=3, space="PSUM") as psp,
            tc.tile_pool(name="po", bufs=1, space="PSUM") as pop,
        ):
            # ---------------- constants ----------------
            eps_t = consts.tile([P, 1], f32)
            nc.vector.memset(eps_t, EPS)
            ones_row = consts.tile([1, 512], f16)
            nc.vector.memset(ones_row, 1.0)
            ident = consts.tile([P, P], f16)
            make_identity(nc, ident[:])
            ocn = consts.tile([P, 1], f16)
            nc.vector.memset(ocn, -1.0 / C)
            ocp = consts.tile([P, 1], f16)
            nc.vector.memset(ocp, 1.0 / C)

            # prefetch activation tables so the LN1 rows chain and the
            # first exp/gelu don't eat 1.3us table loads on the critical path
            warm = consts.tile([1, 8], f32, name="warm")
            for fn in (AF.Gelu, AF.Exp, AF.Sqrt):
                nc.scalar.activation(out=warm[0:1, 0:1], in_=eps_t[0:1, 0:1],
                                     func=fn)

            def load_chunked(dst, src_re, n_chunk):
                for c in range(n_chunk):
                    nc.sync.dma_start(out=dst[:, c], in_=src_re[:, c])

            # prologue DMAs (xT + wqk first: they gate everything)
            xT_sb = acts.tile([P, CC, N], f16, tag="xT")
            load_chunked(xT_sb, xT_d.rearrange("(ci p) n -> p ci n", p=P), CC)
            wqk_sb = consts.tile([P, CC, 2 * C], f16, tag="wqk")
            load_chunked(wqk_sb, wqk_d.rearrange("(ci p) d -> p ci d", p=P), CC)
            rqk_sb = consts.tile([2, 2 * C], f16)
            nc.sync.dma_start(out=rqk_sb, in_=rqk_d[:, :])
            wv_sb = consts.tile([P, CC, VW], f16, tag="wv")
            load_chunked(wv_sb, wv_d.rearrange("(ci p) d -> p ci d", p=P), CC)
            rv_sb = consts.tile([2, VW], f16)
            nc.sync.dma_start(out=rv_sb, in_=rv_d[:, :])
            kcT_sb = acts.tile([P, NT, N], f16, tag="kcT")
            load_chunked(kcT_sb, kcT_d.rearrange("(mi p) n -> p mi n", p=P), NT)

            # ---------------- phase A: raw LN1 stats (feature-major) --------
            # -mu accumulates in a psp slot, E[x^2] in the pop slot (the pop
            # ring is idle until attention starts)
            mu_ps = psp.tile([1, N], f32, tag="ps", name="mu_ps")
            esq_ps = pop.tile([1, N], f32, tag="po", name="esq_ps")
            for ci in range(CC):
                sq = tp.tile([P, N], f16, tag="ms", bufs=2, name="sq")
                nc.vector.tensor_mul(sq[:], xT_sb[:, ci, :], xT_sb[:, ci, :])
                for nj in range(2):
                    sl = slice(nj * 512, (nj + 1) * 512)
                    nc.tensor.matmul(mu_ps[:, sl], lhsT=ocn[:],
                                     rhs=xT_sb[:, ci, sl],
                                     start=(ci == 0), stop=(ci == CC - 1))
                    nc.tensor.matmul(esq_ps[:, sl], lhsT=ocp[:],
                                     rhs=sq[:, sl],
                                     start=(ci == 0), stop=(ci == CC - 1))

            # ---------------- phase B helper: one qk tile ----------------
            qkT = acts.tile([P, DT, N], f16, tag="qkT")

            def qk_raw(t):
                ps = psp.tile([P, N], f32, tag="ps", name="ps_qk")
                for nj in range(2):
                    sl = slice(nj * 512, (nj + 1) * 512)
                    for ci in range(CC):
                        nc.tensor.matmul(
                            ps[:, sl],
                            lhsT=wqk_sb[:, ci, t * P:(t + 1) * P],
                            rhs=xT_sb[:, ci, sl],
                            start=(ci == 0), stop=False)
                return ps

            def qk_fin(t, ps):
                for nj in range(2):
                    sl = slice(nj * 512, (nj + 1) * 512)
                    nc.tensor.matmul(ps[:, sl],
                                     lhsT=rqk_sb[:, t * P:(t + 1) * P],
                                     rhs=fixrows[:, sl],
                                     start=False, stop=True)
                nc.vector.tensor_tensor(out=qkT[:, t, :], in0=ps[:],
                                        in1=rs1_b[:], op=OP.mult)

            def qk_tile(t):
                qk_fin(t, qk_raw(t))

            # raw qk for tiles 0 and 6 runs while the stats rows chain (ACT/
            # DVE) drains, so the PE never waits on LN1
            ps_t0 = qk_raw(0)
            ps_t6 = qk_raw(CC)

            # rows chain: var = esq - mu^2 ; rstd = exp(-.5 ln(var+eps));
            # std = exp(+.5 ln(var+eps)); fixrows = [-mu ; std]
            fixrows = consts.tile([2, N], f16, name="fixrows")
            nc.scalar.copy(out=fixrows[0:1, :], in_=mu_ps[:])
            musq = tp.tile([1, N], f16, tag="rrow", bufs=2, name="musq")
            nc.vector.tensor_mul(musq[:], mu_ps[:], fixrows[0:1, :])
            var_sb = tp.tile([1, N], f16, tag="rrow", bufs=2, name="var_sb")
            nc.vector.tensor_tensor(out=var_sb[:], in0=esq_ps[:],
                                    in1=musq[:], op=OP.subtract)
            # std = sqrt(var+eps); rstd = 1/std via the wide [8,128] DVE
            # reciprocal (Sqrt keeps ACT in the already-warm sqrt table set;
            # Ln/Exp here would ping-pong table loads every use)
            std_row = tp.tile([1, N], f16, tag="rrow", bufs=2, name="std_row")
            nc.scalar.activation(out=std_row[:], in_=var_sb[:], func=AF.Sqrt,
                                 bias=eps_t[0:1, 0:1])
            # engines can only write partition offsets 0/32/64/96, so DMA the
            # std row into fixrows row 1
            nc.sync.dma_start(out=fixrows[1:2, :], in_=std_row[:])
            sd8 = tp.tile([NT, P], f16, tag="s8", bufs=2, name="sd8")
            for a in range(NT):
                nc.sync.dma_start(out=sd8[a:a + 1, :],
                                  in_=std_row[0:1, a * P:(a + 1) * P])
            rsd8 = tp.tile([NT, P], f16, tag="rec8", bufs=2, name="rsd8")
            with nc.allow_low_precision(reason="ln scale applied in f16"):
                nc.vector.reciprocal(out=rsd8[:], in_=sd8[:])
            rstd_row = consts.tile([1, N], f16, name="rstd_row")
            for a in range(NT):
                nc.sync.dma_start(out=rstd_row[0:1, a * P:(a + 1) * P],
                                  in_=rsd8[a:a + 1, :])

            # rstd broadcast row -> [128, N] (for feature-major qk evac)
            rsb_ps = psp.tile([P, N], f32, tag="ps", name="rsb_ps")
            for nj in range(2):
                sl = slice(nj * 512, (nj + 1) * 512)
                nc.tensor.matmul(rsb_ps[:, sl], lhsT=ones_row[:, 0:P],
                                 rhs=rstd_row[:, sl], start=True, stop=True)
            rs1_b = consts.tile([P, N], f16, name="rs1_b")
            nc.scalar.copy(out=rs1_b[:], in_=rsb_ps[:])

            qk_fin(0, ps_t0)
            qk_fin(CC, ps_t6)

            # rstd as per-token columns [128, NT] (for token-major v evac):
            # PE-transpose of the [8,128] reciprocal tile.
            rstd_cols = consts.tile([P, NT], f32, name="rstd_cols")

            def make_rstd_cols():
                rsc_ps = psp.tile([P, NT], f16, tag="ps", name="rsc_ps")
                nc.tensor.transpose(rsc_ps[:], rsd8[:], ident[0:NT, 0:NT])
                nc.vector.tensor_copy(rstd_cols[:], rsc_ps[:])

            # ---------------- phase C helper: one v tile ----------------
            v_aug = acts.tile([P, NT, VW], f16, tag="v_aug")

            def v_tile(mi):
                ps = psp.tile([P, VW], f32, tag="ps", name="ps_v")
                for c0, c1 in ((0, 512), (512, VW)):
                    for ci in range(CC):
                        nc.tensor.matmul(
                            ps[:, c0:c1],
                            lhsT=xT_sb[:, ci, mi * P:(mi + 1) * P],
                            rhs=wv_sb[:, ci, c0:c1],
                            start=(ci == 0), stop=False)
                    nc.tensor.matmul(ps[:, c0:c1],
                                     lhsT=fixrows[:, mi * P:(mi + 1) * P],
                                     rhs=rv_sb[:, c0:c1],
                                     start=False, stop=True)
                nc.scalar.activation(out=v_aug[:, mi, :], in_=ps[:],
                                     func=AF.Copy,
                                     scale=rstd_cols[:, mi:mi + 1])

            # ---------------- phase D: attention per head ----------------
            attn_oT = acts.tile([P, CC, N], f16, tag="attn_oT")
            NS = NT // 2                      # 4 slabs of 2 token tiles

            def head(h, filler=None):
                t_q, off = h // 2, (h % 2) * HS
                t_k = CC + h // 2
                po = pop.tile([HS + 1, N], f32, tag="po", name="po")
                exp_sl = [None] * NS

                for s in range(NS):
                    ms = tp.tile([P, 2, N], f16, tag="ms", bufs=2, name="ms")
                    for q in range(2):
                        mi = 2 * s + q
                        ps = psp.tile([P, N], f32, tag="ps", name="ps_sc")
                        for nj in range(2):
                            sl = slice(nj * 512, (nj + 1) * 512)
                            nc.tensor.matmul(
                                ps[:, sl],
                                lhsT=qkT[off:off + HS, t_k,
                                         mi * P:(mi + 1) * P],
                                rhs=qkT[off:off + HS, t_q, sl],
                                start=True, stop=True)
                        nc.vector.tensor_mul(ms[:, q, :], ps[:],
                                             kcT_sb[:, mi, :])
                    expT = tp.tile([P, 2, N], f16, tag="expT", bufs=4,
                                   name="expT")
                    nc.scalar.activation(out=expT[:], in_=ms[:], func=AF.Exp)
                    exp_sl[s] = expT

                if filler is not None:
                    filler()

                for s in range(NS):
                    for q in range(2):
                        mi = 2 * s + q
                        for nj in range(2):
                            sl = slice(nj * 512, (nj + 1) * 512)
                            nc.tensor.matmul(
                                po[:, sl],
                                lhsT=v_aug[:, mi, h * (HS + 1):
                                           (h + 1) * (HS + 1)],
                                rhs=exp_sl[s][:, q, sl],
                                start=(mi == 0), stop=(mi == NT - 1))

                # evacuate the head output + softmax sums. even head: one ACT
                # copy of [65, N] (sums land on partition 64, a legal offset
                # to read from). odd head: head rows to partitions 64..127
                # plus a separate sums-row copy. Then the sums row is DMA-
                # reshaped to [8, 128] so the DVE reciprocal runs wide, and
                # DMA'd back to a row for the PE ones-matmul broadcast.
                if h % 2 == 0:
                    att = tp.tile([HS + 1, N], f16, tag="att_e", bufs=2,
                                  name="att_e")
                    nc.scalar.copy(out=att[:], in_=po[:])
                    srow = att[HS:HS + 1, :]
                else:
                    att = tp.tile([P, N], f16, tag="att_o", bufs=2,
                                  name="att_o")
                    nc.scalar.copy(out=att[HS:P, :], in_=po[0:HS, :])
                    sums = tp.tile([1, N], f16, tag="sums_o", bufs=1,
                                   name="sums_o")
                    nc.scalar.copy(out=sums[:], in_=po[HS:HS + 1, :])
                    srow = sums[:]
                s8 = tp.tile([NT, P], f16, tag="s8", bufs=2, name="s8")
                for a in range(NT):
                    nc.sync.dma_start(out=s8[a:a + 1, :],
                                      in_=srow[0:1, a * P:(a + 1) * P])
                rec8 = tp.tile([NT, P], f16, tag="rec8", bufs=2, name="rec8")
                with nc.allow_low_precision(reason="attn weights are f16"):
                    nc.vector.reciprocal(out=rec8[:], in_=s8[:])
                rrow = tp.tile([1, N], f16, tag="rrow", bufs=2, name="rrow")
                for a in range(NT):
                    nc.sync.dma_start(out=rrow[0:1, a * P:(a + 1) * P],
                                      in_=rec8[a:a + 1, :])
                return att, rrow

            def finish_pair(j, ae, ao, last=False):
                att_e, rrow_e = ae
                att_o, rrow_o = ao
                dps = psp.tile([P, N], f32, tag="ps", name="den_ps")
                for nj in range(2):
                    sl = slice(nj * 512, (nj + 1) * 512)
                    nc.tensor.matmul(dps[0:HS, sl], lhsT=ones_row[:, 0:HS],
                                     rhs=rrow_e[:, sl], start=True, stop=True)
                    nc.tensor.matmul(dps[HS:P, sl], lhsT=ones_row[:, 0:HS],
                                     rhs=rrow_o[:, sl], start=True, stop=True)
                if last:
                    nc.vector.tensor_mul(attn_oT[0:HS, j, :], att_e[0:HS, :],
                                         dps[0:HS, :])
                    nc.vector.tensor_mul(attn_oT[HS:P, j, :], att_o[HS:P, :],
                                         dps[HS:P, :])
                    return
                den = tp.tile([P, N], f16, tag="den", bufs=2, name="den")
                if j % 2 == 0:
                    nc.scalar.copy(out=den[:], in_=dps[:])
                else:
                    nc.vector.tensor_copy(den[:], dps[:])
                nc.gpsimd.tensor_mul(attn_oT[0:HS, j, :], att_e[0:HS, :],
                                     den[0:HS, :])
                nc.gpsimd.tensor_mul(attn_oT[HS:P, j, :], att_o[HS:P, :],
                                     den[HS:P, :])

            # late weight loads (emitted mid-D; DMA only, no engine work)
            wp_sb = consts.tile([P, CC, C], f16, tag="wp")
            w1_sb = consts.tile([P, CC, C], f16, tag="w1")
            b1_sb = consts.tile([P, CC], f32)
            w2_sb = consts.tile([P, CC, C], f16, tag="w2")
            b2_row = consts.tile([1, C], f16)

            def late_loads():
                load_chunked(wp_sb, wp_d.rearrange("(ci p) d -> p ci d", p=P),
                             CC)
                load_chunked(w1_sb, w1_d.rearrange("(ci p) d -> p ci d", p=P),
                             CC)
                nc.sync.dma_start(out=b1_sb,
                                  in_=b1_d.rearrange("(t p) -> p t", p=P))
                load_chunked(w2_sb, w2_d.rearrange("(ci p) d -> p ci d", p=P),
                             CC)
                nc.sync.dma_start(out=b2_row, in_=b2_d[None, :])

            # ---- emission schedule: B/C interleaved into D as PE filler ----
            # pair j consumes qk tiles (j, CC+j): the q tile is emitted two
            # heads ahead (even head), the k tile one head ahead (odd head),
            # so every head gets a PE filler block between its scores and
            # attnv. finish_pair(j) (the den broadcast + normalize) is
            # deferred into head 2j+2's filler so its PE passes never wait
            # on the sums DMA/reciprocal round trip.
            qk_tile(0)
            qk_tile(CC)

            # dense B then C: uninterrupted same-shape PE streams ramp the
            # p-state; interleaving them into D measured ~15% slower per pass
            for t in list(range(1, CC)) + list(range(CC + 1, DT)):
                qk_tile(t)
            make_rstd_cols()
            for mi in range(NT):
                v_tile(mi)
            late_loads()

            atts = {}
            for h in range(H):
                j = h // 2

                def fill(h=h, j=j):
                    if h >= 2 and h % 2 == 0:
                        finish_pair(j - 1, atts[h - 2], atts[h - 1])

                atts[h] = head(h, filler=fill)

            # ---------------- phase E: proj + residual + LN2 -> znT ----------
            # proj(ni=0)'s first 5 channel chunks are emitted BEFORE the last
            # pair's normalize so the PE chews on them instead of stalling on
            # the sums/reciprocal round trip; the ci=5 chunk (which needs that
            # pair's output) closes the accumulation after.
            y_sb = acts.tile([P, NT, C], f32, tag="qkT")
            zn_all = acts.tile([P, NT, C], f16, tag="v_aug")
            znT = acts.tile([P, CC, N], f16, tag="kcT")

            def proj_chunks(ps, ni, cis):
                for c0, c1 in ((0, 512), (512, C)):
                    for ci in cis:
                        nc.tensor.matmul(
                            ps[:, c0:c1],
                            lhsT=attn_oT[:, ci, ni * P:(ni + 1) * P],
                            rhs=wp_sb[:, ci, c0:c1],
                            start=(ci == 0), stop=(ci == CC - 1))

            ps0 = psp.tile([P, C], f32, tag="ps", name="ps_pj0")
            proj_chunks(ps0, 0, range(CC - 1))
            finish_pair(CC - 1, atts[H - 2], atts[H - 1], last=True)

            ln_rows = []
            for ni in range(NT):
                if ni == 0:
                    ps = ps0
                    proj_chunks(ps, 0, [CC - 1])
                else:
                    ps = psp.tile([P, C], f32, tag="ps", name="ps_pj")
                    proj_chunks(ps, ni, range(CC))
                x_t = tp.tile([P, C], f16, tag="x_in", bufs=2)
                nc.sync.dma_start(out=x_t, in_=x_d[ni * P:(ni + 1) * P, :])
                nc.vector.tensor_add(y_sb[:, ni, :], x_t[:], ps[:])
                # LN2 stats (token-major, bn_stats on DVE)
                stats = tp.tile([P, 3, nc.vector.BN_STATS_DIM], f32,
                                tag="ln_stats", bufs=2)
                for s3 in range(3):
                    nc.vector.bn_stats(out=stats[:, s3, :],
                                       in_=y_sb[:, ni, s3 * 256:(s3 + 1) * 256])
                mv = tp.tile([P, nc.vector.BN_AGGR_DIM], f32, tag="ln_mv",
                             bufs=NT)
                nc.vector.bn_aggr(out=mv, in_=stats)
                lv2 = tp.tile([P, 1], f32, tag="ln_std", bufs=2)
                nc.scalar.activation(out=lv2, in_=mv[:, 1:2], func=AF.Sqrt,
                                     bias=eps_t[:, 0:1], scale=1.0)
                rstd = tp.tile([P, 1], f32, tag="ln_rstd", bufs=NT)
                nc.vector.reciprocal(out=rstd, in_=lv2)
                ln_rows.append((mv, rstd))
            # zn after all proj so the DVE never starves the proj PSUM ring;
            # transposes trail zn by one ni
            for ni in range(NT):
                mv, rstd = ln_rows[ni]
                nc.vector.tensor_scalar(out=zn_all[:, ni, :],
                                        in0=y_sb[:, ni, :],
                                        scalar1=mv[:, 0:1], scalar2=rstd[:],
                                        op0=OP.subtract, op1=OP.mult)
            for ni in range(NT):
                for ci in range(CC):
                    pt = psp.tile([P, P], f16, tag="ps", name="pt")
                    nc.tensor.transpose(pt[:], zn_all[:, ni, ci * P:(ci + 1) * P],
                                        ident[:])
                    nc.scalar.copy(out=znT[:, ci, ni * P:(ni + 1) * P],
                                   in_=pt[:])

            # ---------------- phase F: fc1 + exact gelu -> hgT ----------------
            hgT = acts.tile([P, CC, N], f16, tag="xT")
            for t in range(CC):
                ps = psp.tile([P, N], f32, tag="ps", name="ps_f1")
                for nj in range(2):
                    sl = slice(nj * 512, (nj + 1) * 512)
                    for ci in range(CC):
                        nc.tensor.matmul(
                            ps[:, sl],
                            lhsT=w1_sb[:, ci, t * P:(t + 1) * P],
                            rhs=znT[:, ci, sl],
                            start=(ci == 0), stop=(ci == CC - 1))
                nc.scalar.activation(out=hgT[:, t, :], in_=ps[:],
                                     func=AF.Gelu, bias=b1_sb[:, t:t + 1])

            # ---------------- phase G: fc2 + residual -> out ----------------
            for ni in range(NT):
                ps = psp.tile([P, C], f32, tag="ps", name="ps_f2")
                for c0, c1 in ((0, 512), (512, C)):
                    for ci in range(CC):
                        nc.tensor.matmul(
                            ps[:, c0:c1],
                            lhsT=hgT[:, ci, ni * P:(ni + 1) * P],
                            rhs=w2_sb[:, ci, c0:c1],
                            start=(ci == 0), stop=False)
                    nc.tensor.matmul(ps[:, c0:c1], lhsT=ones_row[:, 0:P],
                                     rhs=b2_row[:, c0:c1], start=False,
                                     stop=True)
                o_t = tp.tile([P, C], f32, tag="o_out", bufs=2)
                nc.vector.tensor_add(o_t[:], y_sb[:, ni, :], ps[:])
                nc.sync.dma_start(out=out_d[ni * P:(ni + 1) * P, :], in_=o_t[:])

    nc.compile()
    return nc


_NC = None
LAST_RESULTS = None
TRACE = False


def _prep_weights(inputs):
    qkv_w = np.asarray(inputs["qkv_w"], np.float64)
    proj_w = np.asarray(inputs["proj_w"], np.float64)
    fc1_w = np.asarray(inputs["fc1_w"], np.float64)
    fc2_w = np.asarray(inputs["fc2_w"], np.float64)
    ln1_w = np.asarray(inputs["ln1_w"], np.float64)
    ln1_b = np.asarray(inputs["ln1_b"], np.float64)
    ln2_w = np.asarray(inputs["ln2_w"], np.float64)
    ln2_b = np.asarray(inputs["ln2_b"], np.float64)

    Wqk = qkv_w[:2 * C] * ln1_w[None, :]
    bqk = ln1_b @ qkv_w[:2 * C].T
    wqkT = Wqk.T.copy()
    wqkT[:, :C] *= SCALE
    bqk = bqk.copy()
    bqk[:C] *= SCALE
    rows_qk = np.stack([wqkT.sum(axis=0), bqk])          # [2, 2C]

    Wv = qkv_w[2 * C:] * ln1_w[None, :]
    bv = ln1_b @ qkv_w[2 * C:].T
    wvT = Wv.T                                            # [c, C]
    wv_aug = np.zeros((C, VW), np.float64)
    bv_aug = np.zeros((VW,), np.float64)
    for h in range(H):
        base = h * (HS + 1)
        wv_aug[:, base:base + HS] = wvT[:, h * HS:(h + 1) * HS]
        bv_aug[base:base + HS] = bv[h * HS:(h + 1) * HS]
        bv_aug[base + HS] = 1.0
    rows_v = np.stack([wv_aug.sum(axis=0), bv_aug])      # [2, VW]

    fc1T = (fc1_w * ln2_w[None, :]).T.copy()
    fc1_b_eff = ln2_b @ fc1_w.T + np.asarray(inputs["fc1_b"], np.float64)

    return {
        "wqkT": wqkT.astype(np.float16),
        "rows_qk": rows_qk.astype(np.float16),
        "wvT": wv_aug.astype(np.float16),
        "rows_v": rows_v.astype(np.float16),
        "projT": proj_w.T.astype(np.float16).copy(),
        "fc1T": fc1T.astype(np.float16),
        "fc1_b": fc1_b_eff.astype(np.float32),
        "fc2T": fc2_w.T.astype(np.float16).copy(),
        "fc2_b": np.asarray(inputs["fc2_b"], np.float32).astype(np.float16),
    }


def kernel(**inputs):
    global _NC, LAST_RESULTS
    if _NC is None:
        _NC = build_kernel()

    jf = np.ascontiguousarray(np.asarray(inputs["joint_feature"], np.float32))
    kc = np.asarray(inputs["k_conn"], np.float32)
    shared = _prep_weights(inputs)
    x_pb = jf + np.asarray(inputs["proj_b"], np.float32)[None, None, :]

    in_maps = []
    for b in range(B):
        m = dict(shared)
        m["x"] = np.ascontiguousarray(x_pb[b]).astype(np.float16)
        m["xT"] = np.ascontiguousarray(jf[b].T).astype(np.float16)
        m["kcT"] = np.ascontiguousarray(kc[b].T).astype(np.float16)
        in_maps.append(m)

    res = run_bass_kernel_spmd(_NC, in_maps, core_ids=list(range(B)), trace=TRACE)
    LAST_RESULTS = res
    out = np.stack([res.results[b]["out"] for b in range(B)], axis=0)
    return out.astype(np.float32)


if __name__ == "__main__":
    nc = build_kernel()
    print("kernel built OK")
